# revision 9
# baseline (speedup 1.0000x reference)
"""Distributed Trainium2 kernel for nn_AddAttention_154618823089.

Computation (see reference):
    q = rope(bf16(hidden @ Wq.T)); k = rope(bf16(hidden @ Wk.T))
    o[b,l] = sum_{j<=l} exp(q_l . k_j / sqrt(DIM))          (no softmax norm)
    out = relu(o @ fc1_w.T + fc1_b) @ fc2_w.T + fc2_b

Sharding: 8 cores = 2 batches x 4 ranks.  Core c handles batch c//4 and the
STRIDED row set {r : r % 4 == c % 4} (1024 rows).  Striding makes the causal
workload identical on every core: each local q-subtile s (128 rows spanning
global rows [512s, 512(s+1))) needs k-blocks 0..s, so every core runs the same
36-block loop; the causal boundary needs one [128,512] additive mask that is
the same for every s (only depends on the core's rank).

Per-core flow:
  - project k for own rows (bf16 matmul, d-on-partitions layout), RoPE in-place
  - AllGather k within the 4-core batch group (overlaps with q proj + RoPE)
  - 36 score blocks [128q x 512k], contraction over d=1024 (8 psum matmuls),
    fused exp + row-sum on scalar engine (accum_out)
  - tiny MLP: broadcast o via K=1 matmul, relu(o*w1+b1) via activation
    scale/bias, final K=17 matmul with bias row folded in
"""

import sys
import types

import numpy as np
from ml_dtypes import bfloat16

import concourse.bacc as bacc
import concourse.bass as bass
import concourse.mybir as mybir
import concourse.tile as tile
from concourse.bass_utils import run_bass_kernel_spmd


def _install_ntff_hook():
    """The container's antenv lacks axon_hooks; provide it so trace=True can
    capture NTFF profiles (exec_time_ns) through the axon PJRT library."""
    if "antenv.axon_hooks" in sys.modules:
        return
    try:
        sys.path.insert(0, "/root/.axon_site/trn_agent_boot")
        import trn_boot

        mod = types.ModuleType("antenv.axon_hooks")
        _h = {"hook": None}
        mod.set_axon_ntff_profile_hook = lambda h: _h.__setitem__("hook", h)
        mod.get_axon_ntff_profile_hook = lambda: _h["hook"]
        sys.modules["antenv.axon_hooks"] = mod
        import antenv

        antenv.axon_hooks = mod
        mod.set_axon_ntff_profile_hook(
            trn_boot._ntff_profile_via_ctypes("/opt/axon/libaxon_pjrt.so"))
    except Exception:
        pass


_install_ntff_hook()

B, L, DIM, INNER = 2, 4096, 1024, 16
ROPE_BASE = 32.0
NCORES = 8
RANKS = 4              # cores per batch group
RLOC = L // RANKS      # local rows per core (1024)
NSUB = RLOC // 128     # q subtiles per core (8)
NDT = DIM // 128       # d tiles (8)
SCALE = 1.0 / float(np.sqrt(DIM))
MASK_NEG = -1.0e6
F32 = mybir.dt.float32
BF16 = mybir.dt.bfloat16

_NC_CACHE = {}


def _build_nc():
    nc = bacc.Bacc("TRN2", target_bir_lowering=False, debug=False,
                   num_devices=NCORES)

    # Per-core inputs (d-major layouts so matmul operands DMA in naturally).
    hT = nc.dram_tensor("hT", [DIM, RLOC], BF16, kind="ExternalInput")
    wqT = nc.dram_tensor("wqT", [DIM, DIM], BF16, kind="ExternalInput")
    wkT = nc.dram_tensor("wkT", [DIM, DIM], BF16, kind="ExternalInput")
    cosh = nc.dram_tensor("cosh", [DIM // 2, RLOC], BF16, kind="ExternalInput")
    sinh = nc.dram_tensor("sinh", [DIM // 2, RLOC], BF16, kind="ExternalInput")
    maskin = nc.dram_tensor("maskin", [128, 512], F32, kind="ExternalInput")
    fc1w = nc.dram_tensor("fc1w", [INNER, 1], F32, kind="ExternalInput")
    fc1b = nc.dram_tensor("fc1b", [INNER, 1], F32, kind="ExternalInput")
    w2aug = nc.dram_tensor("w2aug", [INNER + 1, DIM], BF16, kind="ExternalInput")
    onesrow = nc.dram_tensor("onesrow", [1, RLOC], BF16, kind="ExternalInput")
    out_d = nc.dram_tensor("out", [RLOC, DIM], F32, kind="ExternalOutput")

    # Internal DRAM: k bounce for the collective, gathered K, o transpose hop.
    kT_bounce = nc.dram_tensor("kT_bounce", [DIM, RLOC], BF16)
    G = nc.dram_tensor("G", [RANKS * DIM, RLOC], BF16)
    o_dram = nc.dram_tensor("o_dram", [NSUB, 128], F32)

    groups = [[0, 1, 2, 3], [4, 5, 6, 7]]

    with tile.TileContext(nc) as tc:
        with (
            tc.tile_pool(name="big", bufs=1) as big,
            tc.tile_pool(name="tmp", bufs=4) as tmp,
            tc.tile_pool(name="es", bufs=2) as esp,
            tc.tile_pool(name="rsp", bufs=2) as rsp,
            tc.tile_pool(name="osb", bufs=2) as osbp,
            tc.tile_pool(name="ps", bufs=3, space="PSUM") as pps,
            tc.tile_pool(name="pso", bufs=1, space="PSUM") as ppo,
            tc.tile_pool(name="psb", bufs=1, space="PSUM") as ppb,
        ):
            # ---- constants / weights in SBUF ----
            h_sb = big.tile([128, NDT, RLOC], BF16, tag="h")
            nc.sync.dma_start(h_sb[:], hT.rearrange("(t p) r -> p t r", p=128))
            wk_sb = big.tile([128, NDT, DIM], BF16, tag="wk")
            nc.sync.dma_start(wk_sb[:], wkT.rearrange("(t p) d -> p t d", p=128))
            wq_sb = big.tile([128, NDT, DIM], BF16, tag="wq")
            nc.sync.dma_start(wq_sb[:], wqT.rearrange("(t p) d -> p t d", p=128))
            cos_sb = big.tile([128, NDT // 2, RLOC], BF16, tag="cos")
            nc.sync.dma_start(cos_sb[:], cosh.rearrange("(t p) r -> p t r", p=128))
            sin_sb = big.tile([128, NDT // 2, RLOC], BF16, tag="sin")
            nc.sync.dma_start(sin_sb[:], sinh.rearrange("(t p) r -> p t r", p=128))
            mask_sb = big.tile([128, 512], F32, tag="mask")
            nc.sync.dma_start(mask_sb[:], maskin[:])
            fc1w_sb = big.tile([INNER, 1], F32, tag="fc1w")
            nc.sync.dma_start(fc1w_sb[:], fc1w[:])
            fc1b_sb = big.tile([INNER, 1], F32, tag="fc1b")
            nc.sync.dma_start(fc1b_sb[:], fc1b[:])
            w2_sb = big.tile([INNER + 1, DIM], BF16, tag="w2")
            nc.sync.dma_start(w2_sb[:], w2aug[:])
            ones1 = big.tile([1, INNER], F32, tag="ones1")
            nc.vector.memset(ones1[:], 1.0)

            def project_rope(w_sb, dest_tag):
                """proj = bf16(hidden @ W.T) in [d partitions, rows] layout,
                then RoPE in place.  Returns the SBUF tile [128, NDT, RLOC]."""
                proj = big.tile([128, NDT, RLOC], BF16, tag=dest_tag)
                for rt in range(RLOC // 512):
                    for do in range(NDT):
                        ps = pps.tile([128, 512], F32, tag="ps")
                        for di in range(NDT):
                            nc.tensor.matmul(
                                ps[:],
                                w_sb[:, di, 128 * do:128 * (do + 1)],
                                h_sb[:, di, 512 * rt:512 * (rt + 1)],
                                start=(di == 0), stop=(di == NDT - 1),
                            )
                        # round to bf16 (reference casts q/k to bf16 here)
                        nc.vector.tensor_copy(
                            proj[:, do, 512 * rt:512 * (rt + 1)], ps[:])
                # RoPE in place, half-dim pairs (dt, dt+4):
                #   lo' = lo*cos - hi*sin ; hi' = hi*cos + lo*sin
                for dt in range(NDT // 2):
                    cm = cos_sb[:, dt, :]
                    sm = sin_sb[:, dt, :]
                    lo = proj[:, dt, :]
                    hi = proj[:, dt + NDT // 2, :]
                    ta = tmp.tile([128, RLOC], BF16, tag="ta")
                    tb = tmp.tile([128, RLOC], BF16, tag="tb")
                    td = tmp.tile([128, RLOC], BF16, tag="td")
                    nc.vector.tensor_mul(ta[:], lo, cm)
                    nc.vector.tensor_mul(tb[:], lo, sm)
                    nc.vector.tensor_mul(td[:], hi, sm)
                    nc.vector.tensor_sub(lo, ta[:], td[:])
                    nc.vector.tensor_mul(ta[:], hi, cm)
                    nc.vector.tensor_add(hi, ta[:], tb[:])
                return proj

            # ---- k first: project, rope, bounce out, all-gather ----
            k_rope = project_rope(wk_sb, "krope")
            for dt in range(NDT):
                nc.sync.dma_start(kT_bounce[128 * dt:128 * (dt + 1), :],
                                  k_rope[:, dt, :])
            nc.gpsimd.collective_compute(
                "AllGather",
                mybir.AluOpType.bypass,
                replica_groups=groups,
                ins=[kT_bounce.ap().opt()],
                outs=[G.ap().opt()],
            )

            # ---- q: project + rope (overlaps with the collective) ----
            q_rope = project_rope(wq_sb, "qrope")

            # ---- gathered K -> SBUF, split by k-block for pipelining ----
            # K_sb layout: [128 (d in tile), NDT, RANKS, RLOC]
            k_sb = big.tile([128, NDT, RANKS, RLOC], BF16, tag="ksb")
            g_r = G.rearrange("(r t p) j -> p t r j", r=RANKS, t=NDT, p=128)
            for kb in range(NSUB):
                for r in range(RANKS):
                    nc.sync.dma_start(k_sb[:, :, r, 128 * kb:128 * (kb + 1)],
                                      g_r[:, :, r, 128 * kb:128 * (kb + 1)])

            # ---- causal score blocks with fused exp + row-sum ----
            o_sb = big.tile([128, NSUB], F32, tag="o")
            for s in range(NSUB):
                rs = rsp.tile([128, NSUB], F32, tag="rs")
                for kb in range(s + 1):
                    ps = pps.tile([128, 512], F32, tag="ps")
                    for dt in range(NDT):
                        nc.tensor.matmul(
                            ps[:],
                            q_rope[:, dt, 128 * s:128 * (s + 1)],
                            k_sb[:, dt, :, 128 * kb:128 * (kb + 1)],
                            start=(dt == 0), stop=(dt == NDT - 1),
                        )
                    if kb == s:
                        nc.vector.tensor_add(ps[:], ps[:], mask_sb[:])
                    es = esp.tile([128, 512], BF16, tag="es")
                    nc.scalar.activation(
                        es[:], ps[:], mybir.ActivationFunctionType.Exp,
                        scale=SCALE, accum_out=rs[:, kb:kb + 1],
                    )
                nc.vector.reduce_sum(o_sb[:, s:s + 1], rs[:, 0:s + 1],
                                     axis=mybir.AxisListType.X)

            # ---- MLP: out = relu(o @ fc1.T + b1) @ fc2.T + b2 ----
            # o_sb[p, s] is local row 128s+p; hop through DRAM to get row-major
            nc.sync.dma_start(o_dram.rearrange("s p -> p s"), o_sb[:])
            o_row = big.tile([1, RLOC], F32, tag="orow")
            nc.sync.dma_start(o_row[:], o_dram.rearrange("s p -> (s p)")[None, :])

            bc = ppb.tile([INNER, RLOC], F32, tag="bc")
            for h in range(RLOC // 512):
                nc.tensor.matmul(bc[:, 512 * h:512 * (h + 1)], ones1[:],
                                 o_row[:, 512 * h:512 * (h + 1)],
                                 start=True, stop=True)
            z_aug = big.tile([INNER + 1, RLOC], BF16, tag="zaug")
            nc.scalar.activation(z_aug[0:INNER, :], bc[:],
                                 mybir.ActivationFunctionType.Relu,
                                 bias=fc1b_sb[:], scale=fc1w_sb[:])
            nc.sync.dma_start(z_aug[INNER:INNER + 1, :], onesrow[:])

            for t in range(NSUB):
                po = ppo.tile([128, 512], F32, tag="po")
                po2 = ppo.tile([128, 512], F32, tag="po2")
                nc.tensor.matmul(po[:], z_aug[:, 128 * t:128 * (t + 1)],
                                 w2_sb[:, 0:512], start=True, stop=True)
                nc.tensor.matmul(po2[:], z_aug[:, 128 * t:128 * (t + 1)],
                                 w2_sb[:, 512:1024], start=True, stop=True)
                ob = osbp.tile([128, DIM], F32, tag="ob")
                nc.vector.tensor_copy(ob[:, 0:512], po[:])
                nc.vector.tensor_copy(ob[:, 512:1024], po2[:])
                nc.sync.dma_start(out_d[128 * t:128 * (t + 1), :], ob[:])

    nc.compile()
    return nc


def get_nc():
    if "nc" not in _NC_CACHE:
        _NC_CACHE["nc"] = _build_nc()
    return _NC_CACHE["nc"]


def make_in_maps(hidden_states, Wq, Wk, fc1_w, fc1_b, fc2_w, fc2_b):
    hidden_states = np.asarray(hidden_states, dtype=np.float32)
    Wq = np.asarray(Wq, dtype=np.float32)
    Wk = np.asarray(Wk, dtype=np.float32)
    fc1_w = np.asarray(fc1_w, dtype=np.float32)
    fc1_b = np.asarray(fc1_b, dtype=np.float32)
    fc2_w = np.asarray(fc2_w, dtype=np.float32)
    fc2_b = np.asarray(fc2_b, dtype=np.float32)

    wqT = np.ascontiguousarray(Wq.T).astype(bfloat16)
    wkT = np.ascontiguousarray(Wk.T).astype(bfloat16)
    fc1w = np.ascontiguousarray(fc1_w.reshape(INNER, 1))
    fc1b = np.ascontiguousarray(fc1_b.reshape(INNER, 1))
    w2aug = np.concatenate([fc2_w.T, fc2_b[None, :]], axis=0).astype(bfloat16)

    inv_freq = ROPE_BASE ** (-np.arange(0, DIM, 2, dtype=np.float32) / DIM)

    in_maps = []
    for c in range(NCORES):
        b, j = c // RANKS, c % RANKS
        rows = np.arange(RLOC) * RANKS + j             # global row ids
        hT = np.ascontiguousarray(
            hidden_states[b, rows, :].T).astype(bfloat16)
        ang = rows[:, None].astype(np.float32) * inv_freq[None, :]  # [RLOC,512]
        cosh = np.ascontiguousarray(np.cos(ang).T).astype(bfloat16)
        sinh = np.ascontiguousarray(np.sin(ang).T).astype(bfloat16)
        # mask[p, jc*128+t]: allow k-col (jc,t) for q-partition p iff
        # 4t + jc <= 4p + j  (same for every boundary subtile s)
        p = np.arange(128)[:, None, None]
        jc = np.arange(RANKS)[None, :, None]
        t = np.arange(128)[None, None, :]
        allow = (4 * t + jc) <= (4 * p + j)
        maskin = np.where(allow, 0.0, MASK_NEG).astype(np.float32)
        maskin = maskin.reshape(128, 512)
        in_maps.append({
            "hT": hT, "wqT": wqT, "wkT": wkT,
            "cosh": cosh, "sinh": sinh, "maskin": maskin,
            "fc1w": fc1w, "fc1b": fc1b, "w2aug": w2aug,
            "onesrow": np.ones((1, RLOC), dtype=bfloat16),
        })
    return in_maps


def assemble_output(results):
    out = np.empty((B, L, DIM), dtype=np.float32)
    for c in range(NCORES):
        b, j = c // RANKS, c % RANKS
        out[b, j::RANKS, :] = results[c]["out"]
    return out


def run(trace=False, **inputs):
    nc = get_nc()
    in_maps = make_in_maps(**inputs)
    res = run_bass_kernel_spmd(nc, in_maps, core_ids=list(range(NCORES)),
                               trace=trace)
    return assemble_output(res.results), res


def kernel(**inputs) -> np.ndarray:
    out, _ = run(trace=False, **inputs)
    return out
